# revision 21
# baseline (speedup 1.0000x reference)
"""Trainium2 Bass kernel for nn_MinibatchLayer (pairwise L1 minibatch-discrimination layer).

Math (reference):
    norm   = sqrt(sum(theta^2, axis=0))                      # [K,P]
    kernel = theta * (exp(lws)/norm)                         # [D,K,P]
    actv   = einsum('bd,dkp->bkp', x, kernel)                # [B,K,P]
    M[i,k,j] = sum_p |actv[i,k,p] - actv[j,k,p]|  (+1e6 on diag)
    f      = sum_j exp(-M) + bias                            # [B,K]
    out    = concat([x, f], axis=1)                          # [B,D+K]

Distribution: shard K=128 across 8 cores (16 kernels/core). Each core reads the
full x (transposed, bf16) + its theta slice; no collectives. Host assembles
out = [x | f] from per-core f blocks.

Per-core pipeline, using |d| = 2*relu(d) - d and the SYMMETRY of
A[i,j] = exp(-sum_p |a_i - a_j|): only upper-triangle tiles are computed.
For i-chunk c (128 i's on partitions) the j range is [128c, 512), length
L = 512-128c; per k the four chunk tiles are packed into 2.5 PSUM banks:
  bank A = c0 (512 cols), bank B = c1 (384) | c3 (128), bank C = c2 (256).
The c1/c3 packing lets ONE FD=512 matmul per plane reduce both tiles
(5 matmuls instead of 10); plane-0 starts the bank (whole-bank zero
region), ysn (-0.5*sum_p Y) matmuls accumulate last. Diff planes are DVE
tensor_scalar (sub+relu, bf16 4x); c3 planes p<4 go to ScalarE as Relu
activations with bias=-a every k, balancing DVE vs ScalarE. NOTE trn2 PSUM
semantics: start=True arms the whole 2KB zero-region, so a bank shared
by two accumulation groups (the colsum collectors) must be armed by a
single explicit zero matmul, with all groups accumulating start=False. Exp with accum_out gives rowsums; one-hot
colsum matmuls credit f[j] for the super-diagonal tiles.
"""

import os
import numpy as np

B, D, K, P = 512, 2048, 128, 5
N_CORES = 8
KC = K // N_CORES          # 16 kernels per core
BT = KC * P                # 80 (k,p) columns per core
NB = B // 128              # 4 batch chunks
ND = D // 128              # 16 contraction chunks

_cache = {}


def _build():
    from concourse import bacc, tile, mybir

    dt = mybir.dt
    f32, bf16 = dt.float32, dt.bfloat16
    Alu = mybir.AluOpType
    Act = mybir.ActivationFunctionType

    nc = bacc.Bacc("TRN2", target_bir_lowering=False, debug=False,
                   num_devices=N_CORES)

    xT = nc.dram_tensor("xT", [128, ND * B], bf16, kind="ExternalInput").ap()
    theta = nc.dram_tensor("theta", [128, ND * BT], bf16, kind="ExternalInput").ap()
    lws = nc.dram_tensor("lws", [BT, 1], f32, kind="ExternalInput").ap()
    biasc = nc.dram_tensor("biasc", [1, KC], f32, kind="ExternalInput").ap()
    identin = nc.dram_tensor("identin", [128, 128], bf16, kind="ExternalInput").ap()
    pselin = nc.dram_tensor("pselin", [BT, KC], bf16, kind="ExternalInput").ap()
    fout = nc.dram_tensor("fout", [B, KC], f32, kind="ExternalOutput").ap()
    avT_dram = nc.dram_tensor("avT_dram", [BT + KC + 1, B], bf16).ap()
    kcolsin = nc.dram_tensor("kcolsin", [128, KC * KC], bf16,
                             kind="ExternalInput").ap()

    with tile.TileContext(nc) as tc:
        with (
            tc.tile_pool(name="const", bufs=1) as constp,
            tc.tile_pool(name="stage", bufs=2) as stagep,
            tc.tile_pool(name="mps", bufs=2, space="PSUM") as mpsp,
            tc.tile_pool(name="cps", bufs=1, space="PSUM") as cpsp,
            tc.tile_pool(name="db", bufs=4) as dbp,
            tc.tile_pool(name="g", bufs=18) as gp,
        ):
            # ---- constants ----
            identb = constp.tile([128, 128], bf16, tag="identb")
            nc.scalar.dma_start(out=identb[:], in_=identin[:])
            ones_bf = constp.tile([128, 1], bf16, tag="ones_bf")
            nc.vector.memset(ones_bf[:], 1.0)
            ones_f1 = stagep.tile([1, 1], f32, tag="ones_f1")
            nc.vector.memset(ones_f1[:], 1.0)
            ones_row = constp.tile([1, 128], f32, tag="ones_row")
            nc.vector.memset(ones_row[:], 1.0)
            identf = constp.tile([KC, KC], f32, tag="identf")
            nc.vector.tensor_copy(identf[:], identb[0:KC, 0:KC])

            # ---- inputs: theta whole on scalar (lands first, feeds norm);
            # x in two halves on sync (feeds avT matmuls progressively) ----
            thtile = constp.tile([128, ND * BT], bf16, tag="thtile")
            xtile = constp.tile([128, ND * B], bf16, tag="xtile")
            nc.scalar.dma_start(out=thtile[:], in_=theta[:])
            nc.sync.dma_start(out=xtile[:], in_=xT[:])
            xts = [xtile[:, c * B:(c + 1) * B] for c in range(ND)]

            psel = constp.tile([BT, KC], bf16, tag="psel")
            nc.scalar.dma_start(out=psel[:], in_=pselin[:])
            # one-hot-row colsum weights: kcols[:, k*KC+m] = (m == k)
            kcols = constp.tile([128, KC * KC], bf16, tag="kcols")
            nc.scalar.dma_start(out=kcols[:], in_=kcolsin[:])
            # warm up the PE on identity data while inputs stream in, so the
            # avT matmuls run at a higher pstate
            wps = mpsp.tile([128, 128], f32, tag="mA", name="wps")
            for w in range(12):
                nc.tensor.matmul(wps[:], lhsT=identb[:], rhs=identb[:],
                                 start=(w == 0), stop=(w == 11))

            # ---- actv_T(raw) = theta.T @ x.T -> [80, 512]; starts as soon as
            # the first theta/x quarters land -- the weight-norm scale is
            # applied AFTER, to the tiny [80,512] result, so the norm chain
            # fully overlaps the GEMM ----
            avT_ps = mpsp.tile([BT, B], f32, tag="mA", name="avT_ps")
            for c in range(ND):
                nc.tensor.matmul(avT_ps[:], lhsT=thtile[:, c * BT:(c + 1) * BT],
                                 rhs=xts[c],
                                 start=(c == 0), stop=(c == ND - 1))

            # norm^2 row -> column; scale_col = exp(lws - 0.5 ln n2)
            sq_all = stagep.tile([128, ND * BT], bf16, tag="sq_all")
            nc.vector.tensor_mul(sq_all[:], thtile[:], thtile[:])
            n2ps = mpsp.tile([1, BT], f32, tag="mB", name="n2ps")
            for c in range(ND):
                nc.tensor.matmul(n2ps[:], lhsT=ones_bf[:],
                                 rhs=sq_all[:, c * BT:(c + 1) * BT],
                                 start=(c == 0), stop=(c == ND - 1))
            n2sb = stagep.tile([1, BT], f32, tag="n2sb")
            nc.vector.tensor_copy(n2sb[:], n2ps[:])
            n2col_ps = mpsp.tile([BT, 1], f32, tag="mC", name="n2col_ps")
            nc.tensor.matmul(n2col_ps[:], lhsT=n2sb[:], rhs=ones_f1[:])
            lncol = stagep.tile([BT, 1], f32, tag="lncol")
            nc.scalar.activation(lncol[:], n2col_ps[:], Act.Ln)
            lwscol = stagep.tile([BT, 1], f32, tag="lwscol")
            nc.scalar.dma_start(out=lwscol[:], in_=lws[:])
            scale_col = stagep.tile([BT, 1], f32, tag="scale_col")
            nc.scalar.activation(scale_col[:], lncol[:], Act.Exp, scale=-0.5,
                                 bias=lwscol[:])
            avT_bf = constp.tile([BT, B], bf16, tag="avT_bf")
            nc.vector.tensor_scalar(out=avT_bf[:], in0=avT_ps[:],
                                    scalar1=scale_col[:], scalar2=None,
                                    op0=Alu.mult)
            # write rows 0..9 first so broadcast chunk 0 can start immediately
            QR0 = BT // 8
            nc.sync.dma_start(out=avT_dram[0:QR0, :], in_=avT_bf[0:QR0, :])
            nc.scalar.dma_start(out=avT_dram[QR0:BT, :], in_=avT_bf[QR0:BT, :])
            yall = constp.tile([128, BT * B], bf16, tag="yall")
            ysn_all = constp.tile([128, KC * B], bf16, tag="ysn_all")

            # Broadcast triggers: sync + gpsimd queues (both compute-idle).
            # Fine granularity (5 yall rows = one k; 4 ysn rows) so the
            # first tiles start as soon as their slice lands.
            def yall_bcast_k(k):
                first4 = {0: nc.sync, 1: nc.gpsimd, 2: nc.scalar, 3: nc.sync}
                eng = first4.get(k, nc.sync if k % 2 == 0 else nc.gpsimd)
                eng.dma_start(
                    out=yall[:, k * P * B:(k + 1) * P * B].rearrange(
                        "a (b c) -> a b c", b=P),
                    in_=avT_dram[k * P:(k + 1) * P, :].partition_broadcast(128))

            def ysn_bcast_q(h):
                nc.gpsimd.dma_start(
                    out=ysn_all[:, h * 4 * B:(h + 1) * 4 * B].rearrange(
                        "a (b c) -> a b c", b=4),
                    in_=avT_dram[BT + h * 4:BT + (h + 1) * 4,
                                 :].partition_broadcast(128))

            ysn_ps = mpsp.tile([KC, B], f32, tag="mB", name="ysn_ps")
            nc.tensor.matmul(ysn_ps[:], lhsT=psel[:], rhs=avT_bf[:])
            ysn_sb = stagep.tile([KC, B], bf16, tag="ysn_sb")
            nc.vector.tensor_scalar(out=ysn_sb[:], in0=ysn_ps[:],
                                    scalar1=-0.5, scalar2=None, op0=Alu.mult)
            nc.scalar.dma_start(out=avT_dram[BT:BT + KC, :], in_=ysn_sb[:])
            nc.scalar.dma_start(out=avT_dram[BT:BT + KC, :], in_=ysn_sb[:])
            # broadcast order matches k-outer consumption
            yall_bcast_k(0)
            ysn_bcast_q(0)
            yall_bcast_k(1)
            yall_bcast_k(2)
            yall_bcast_k(3)
            ysn_bcast_q(1)
            for k_ in range(4, 8):
                yall_bcast_k(k_)
            ysn_bcast_q(2)
            for k_ in range(8, 12):
                yall_bcast_k(k_)
            ysn_bcast_q(3)
            for k_ in range(12, KC):
                yall_bcast_k(k_)

            # ---- actv (b-major) bf16 via PE transpose: 4 x [128, 80] ----
            avs = []
            for bc in range(NB):
                av_ps = mpsp.tile([128, BT], bf16, tag="mC" if bc % 2 else "mB",
                                  name=f"av_ps{bc}")
                nc.tensor.transpose(av_ps[:], avT_bf[:, bc * 128:(bc + 1) * 128],
                                    identb[0:BT, 0:BT])
                av = constp.tile([128, BT], f32, tag=f"av{bc}")
                nc.vector.tensor_copy(av[:], av_ps[:])
                avs.append(av)
            # negated c3 actv columns: Act-relu bias operand
            navs3 = constp.tile([128, BT], f32, tag="navs3")
            nc.vector.tensor_scalar(out=navs3[:], in0=avs[3][:], scalar1=-1.0,
                                    scalar2=None, op0=Alu.mult)

            # ---- bias tile: [128, KC] = bias - 1 (diagonal correction) ----
            brow = stagep.tile([1, KC], f32, tag="brow")
            nc.scalar.dma_start(out=brow[:], in_=biasc[:])
            bm1 = stagep.tile([1, KC], f32, tag="bm1")
            nc.vector.tensor_scalar(out=bm1[:], in0=brow[:], scalar1=1.0,
                                    scalar2=None, op0=Alu.subtract)
            bps = mpsp.tile([128, KC], f32, tag="mA", name="bps")
            nc.tensor.matmul(bps[:], lhsT=ones_row[:], rhs=bm1[:])
            bbias = constp.tile([128, KC], f32, tag="bbias")
            nc.vector.tensor_copy(bbias[:], bps[:])

            # ---- negated per-(i,k) sums over p (Exp bias columns) ----
            nsas = []
            for bc in range(NB):
                nsa = constp.tile([128, KC], f32, tag=f"nsa{bc}", name=f"nsa{bc}")
                nc.vector.tensor_reduce(
                    nsa[:], avs[bc][:].rearrange("a (b c) -> a b c", c=P),
                    axis=mybir.AxisListType.X, op=Alu.add, negate=True)
                nsas.append(nsa)

            # ---- f row-sum accumulators (Act accum_out partials) ----
            fsbs = [constp.tile([128, KC], f32, tag=f"fsb{bc}", name=f"fsb{bc}")
                    for bc in range(NB)]
            # ---- column-sum collectors: PSUM packed [c0(384)|c2(128)] + [c1(256)]
            zeros512 = constp.tile([128, 512], bf16, tag="zeros512")
            nc.vector.memset(zeros512[:], 0.0)
            collAB_ps = cpsp.tile([KC, 512], f32, tag="collAB", name="collAB_ps")
            collB_ps = cpsp.tile([KC, 256], f32, tag="collB", name="collB_ps")
            coll_dst = [collAB_ps[:, 0:384], collB_ps[:, 0:256],
                        collAB_ps[:, 384:512]]
            colls_sb = [constp.tile([KC, 384], f32, tag="coll0", name="coll0"),
                        constp.tile([KC, 256], f32, tag="coll1", name="coll1"),
                        constp.tile([KC, 128], f32, tag="coll2", name="coll2")]

            pending = []  # (k, c, etile, L)

            nc.tensor.matmul(collAB_ps[:, 0:512], lhsT=identb[:, 0:KC],
                             rhs=zeros512[:], start=True, stop=False,
                             skip_group_check=True)

            def emit_colsum(k, c, etile, L):
                # collAB (c0, c2) was zero-armed above -> pure accumulate;
                # collB (c1) arms on its own first matmul
                st = (k == 0) and c == 1
                nc.tensor.matmul(
                    coll_dst[c][:, 0:L - 128],
                    lhsT=kcols[:, k * KC:(k + 1) * KC],
                    rhs=etile[:, 128:L],
                    start=st, stop=(k == KC - 1),
                    skip_group_check=True)

            def ysl(k, j0):
                return ysn_all[:, k * B + j0:(k + 1) * B]

            def ysl_p(k, p, j0, j1):
                base = (k * P + p) * B
                return yall[:, base + j0:base + j1]

            # ---- main loop: k OUTER; per k three PSUM banks A=c0, B=c1|c3,
            # C=c2.  ysn matmuls init each region (start=True), then ONE
            # FD<=512 matmul per plane accumulates: c1+c3 share their plane
            # matmul via the packed bank.  c3 diff planes alternate onto
            # ScalarE (Relu activation, bias=-a) to offload the DVE.
            for k in range(KC):
                mA = mpsp.tile([128, B], f32, tag="mA")
                mB = mpsp.tile([128, B], f32, tag="mB")
                mC = mpsp.tile([128, B], f32, tag="mC")
                dt0 = dbp.tile([128, P * B], bf16, tag="d0")
                dt13 = dbp.tile([128, P * B], bf16, tag="d13")
                dt2 = dbp.tile([128, P * 256], bf16, tag="d2")

                # c3 planes: p<4 on ScalarE (relu activation, bias=-a) every
                # k, p=4 on DVE -- balances the two engines across all k
                for p in range(4):
                    nc.scalar.activation(
                        dt13[:, p * B + 384:(p + 1) * B],
                        ysl_p(k, p, 384, B), Act.Relu,
                        bias=navs3[:, k * P + p:k * P + p + 1], scale=1.0)
                # DVE diff planes (sub + relu, bf16 4x)
                for p in range(P):
                    nc.vector.tensor_scalar(
                        out=dt0[:, p * B:(p + 1) * B], in0=ysl_p(k, p, 0, B),
                        scalar1=avs[0][:, k * P + p:k * P + p + 1],
                        scalar2=0.0, op0=Alu.subtract, op1=Alu.max)
                    nc.vector.tensor_scalar(
                        out=dt13[:, p * B:p * B + 384], in0=ysl_p(k, p, 128, B),
                        scalar1=avs[1][:, k * P + p:k * P + p + 1],
                        scalar2=0.0, op0=Alu.subtract, op1=Alu.max)
                    if p >= 4:
                        nc.vector.tensor_scalar(
                            out=dt13[:, p * B + 384:(p + 1) * B],
                            in0=ysl_p(k, p, 384, B),
                            scalar1=avs[3][:, k * P + p:k * P + p + 1],
                            scalar2=0.0, op0=Alu.subtract, op1=Alu.max)
                    nc.vector.tensor_scalar(
                        out=dt2[:, p * 256:(p + 1) * 256],
                        in0=ysl_p(k, p, 256, B),
                        scalar1=avs[2][:, k * P + p:k * P + p + 1],
                        scalar2=0.0, op0=Alu.subtract, op1=Alu.max)
                # plane-reduce matmuls: first plane starts (zeroes the whole
                # zero-region/bank), rest accumulate; ysn pieces append last
                for p in range(P):
                    nc.tensor.matmul(mA[:, 0:B], lhsT=identb[:],
                                     rhs=dt0[:, p * B:(p + 1) * B],
                                     start=(p == 0), stop=False,
                                     skip_group_check=True)
                    nc.tensor.matmul(mB[:, 0:B], lhsT=identb[:],
                                     rhs=dt13[:, p * B:(p + 1) * B],
                                     start=(p == 0), stop=False,
                                     skip_group_check=True)
                    nc.tensor.matmul(mC[:, 0:256], lhsT=identb[:],
                                     rhs=dt2[:, p * 256:(p + 1) * 256],
                                     start=(p == 0), stop=False,
                                     skip_group_check=True)
                nc.tensor.matmul(mA[:, 0:B], lhsT=identb[:], rhs=ysl(k, 0),
                                 start=False, stop=True, skip_group_check=True)
                nc.tensor.matmul(mB[:, 0:384], lhsT=identb[:], rhs=ysl(k, 128),
                                 start=False, stop=False, skip_group_check=True)
                nc.tensor.matmul(mB[:, 384:B], lhsT=identb[:], rhs=ysl(k, 384),
                                 start=False, stop=True, skip_group_check=True)
                nc.tensor.matmul(mC[:, 0:256], lhsT=identb[:], rhs=ysl(k, 256),
                                 start=False, stop=True, skip_group_check=True)
                # exp + rowsum accumulate; etiles feed lagged colsums
                e0 = gp.tile([128, B], bf16, tag="g")
                nc.scalar.activation(e0[:, 0:B], mA[:, 0:B], Act.Exp,
                                     scale=-2.0, bias=nsas[0][:, k:k + 1],
                                     accum_out=fsbs[0][:, k:k + 1])
                e1 = gp.tile([128, B], bf16, tag="g")
                nc.scalar.activation(e1[:, 0:384], mB[:, 0:384], Act.Exp,
                                     scale=-2.0, bias=nsas[1][:, k:k + 1],
                                     accum_out=fsbs[1][:, k:k + 1])
                nc.scalar.activation(e1[:, 384:B], mB[:, 384:B], Act.Exp,
                                     scale=-2.0, bias=nsas[3][:, k:k + 1],
                                     accum_out=fsbs[3][:, k:k + 1])
                e2 = gp.tile([128, B], bf16, tag="g")
                nc.scalar.activation(e2[:, 0:256], mC[:, 0:256], Act.Exp,
                                     scale=-2.0, bias=nsas[2][:, k:k + 1],
                                     accum_out=fsbs[2][:, k:k + 1])
                pending.append((k, 0, e0, B))
                pending.append((k, 1, e1, 384))
                pending.append((k, 2, e2, 256))
                # drain lagged colsums to keep PE smooth without blocking on Act
                while len(pending) > 3:
                    emit_colsum(*pending.pop(0))
            while pending:
                emit_colsum(*pending.pop(0))
            nc.vector.tensor_copy(colls_sb[0][:], collAB_ps[:, 0:384])
            nc.vector.tensor_copy(colls_sb[2][:], collAB_ps[:, 384:512])
            nc.vector.tensor_copy(colls_sb[1][:], collB_ps[:, 0:256])

            # ---- assemble fout: rowsums + bias-1 + transposed colsum pieces ----
            for c in range(NB):
                of = gp.tile([128, KC], f32, tag="of")
                nc.vector.tensor_add(of[:], fsbs[c][:], bbias[:])
                if c > 0:
                    # colsum pieces for output chunk c from collectors c' < c
                    acc16 = stagep.tile([KC, 128], f32, tag="acc16")
                    first = True
                    for cp in range(c):
                        off = 128 * (c - cp - 1)
                        piece = colls_sb[cp][:, off:off + 128]
                        if first:
                            nc.vector.tensor_copy(acc16[:], piece)
                            first = False
                        else:
                            nc.vector.tensor_add(acc16[:], acc16[:], piece)
                    tp_ps = mpsp.tile([128, KC], f32, tag="mA", name=f"tp{c}")
                    nc.tensor.transpose(tp_ps[:], acc16[:], identf[:])
                    nc.vector.tensor_add(of[:], of[:], tp_ps[:])
                eng = [nc.sync, nc.scalar][c % 2]
                eng.dma_start(out=fout[c * 128:(c + 1) * 128, :], in_=of[:])

    nc.compile()
    return nc


def _get_program():
    if "nc" not in _cache:
        _cache["nc"] = _build()
    return _cache["nc"]


def kernel(x, theta, log_weight_scale, bias, _trace=False):
    import ml_dtypes
    from concourse.bass_utils import run_bass_kernel_spmd

    x = np.asarray(x, dtype=np.float32)
    theta = np.asarray(theta, dtype=np.float32)
    log_weight_scale = np.asarray(log_weight_scale, dtype=np.float32)
    bias = np.asarray(bias, dtype=np.float32)

    nc = _get_program()

    bf = ml_dtypes.bfloat16
    xTl = np.ascontiguousarray(
        x.T.reshape(ND, 128, B).transpose(1, 0, 2).reshape(128, ND * B)
    ).astype(bf)
    ident = np.eye(128, dtype=np.float32).astype(bf)
    # block selector: row (k,p) -> column k (for per-k sums over p)
    psel = np.repeat(np.eye(KC, dtype=np.float32), P, axis=0).astype(bf)
    # one-hot-row colsum weights: kcols[:, k*KC+m] = (m == k)
    kc_np = np.zeros((128, KC * KC), dtype=np.float32)
    for k_ in range(KC):
        kc_np[:, k_ * KC + k_] = 1.0
    kc_np = kc_np.astype(bf)

    in_maps = []
    for c in range(N_CORES):
        ks = slice(c * KC, (c + 1) * KC)
        th = np.ascontiguousarray(
            theta[:, ks, :].reshape(ND, 128, BT)
            .transpose(1, 0, 2).reshape(128, ND * BT)).astype(bf)
        lw = np.ascontiguousarray(
            log_weight_scale[ks, :].reshape(BT, 1)).astype(np.float32)
        bi = np.ascontiguousarray(bias[ks].reshape(1, KC)).astype(np.float32)
        in_maps.append({"xT": xTl, "theta": th, "lws": lw, "biasc": bi,
                        "identin": ident, "pselin": psel, "kcolsin": kc_np})

    res = run_bass_kernel_spmd(nc, in_maps, list(range(N_CORES)),
                               trace=bool(_trace))
    f = np.concatenate([res.results[c]["fout"] for c in range(N_CORES)], axis=1)
    out = np.concatenate([x, f.astype(np.float32)], axis=1)
    if _trace:
        return out, res
    return out
